# revision 4
# baseline (speedup 1.0000x reference)
"""Child-Sum TreeLSTM over a complete 8-ary tree (depth 6, 299593 nodes) on
8 Trainium2 NeuronCores.

Sharding: the 8 independent subtrees under the root go one-per-core; each core
runs the bottom-up sweep for levels L6 (32768 leaves), L5 (4096) and L4 (512)
of its subtree and returns (c4, h4). The top levels (64+8+1 nodes per subtree
plus the root) run on the host in fp32 after the gather.

Device layout: feature-major ([128 features on partitions, nodes on free dim]),
fp16 data, fp32 PSUM. Children are stored j-major ("block" layout: all 0th
children of a parent group, then all 1st children, ...) so that child-sums are
computed by 8 dense identity-weight matmuls accumulating in PSUM (PE) instead
of vector-engine tree adds, and so the per-child forget-gate matmuls need no
strided access patterns (stride-8 rhs runs ~2.8x slower on the PE).

Engine budget per core: ACT is the bottleneck (~154k activation columns at 1
col/cycle @1.2GHz). Leaf tanh(c) is therefore moved off ACT: leaf c =
sigmoid(i)*tanh(u) lies in (-1,1), where a degree-5 odd polynomial (max err
4e-4) evaluated on the vector engine (+ c^2 on gpsimd) replaces tanh. The
host handles levels with <=64 nodes per subtree exactly in fp32.
"""

import os

import numpy as np

import concourse.bass as bass
import concourse.tile as tile
from concourse import bacc, mybir
from concourse.bass_utils import run_bass_kernel_spmd

F16 = mybir.dt.float16
F32 = mybir.dt.float32
SIG = mybir.ActivationFunctionType.Sigmoid
TANH = mybir.ActivationFunctionType.Tanh
ADD = mybir.AluOpType.add
MULT = mybir.AluOpType.mult

BRANCH = 8
DEPTH = 6
MEM = 128
IN_DIM = 128
N_NODES = (BRANCH ** (DEPTH + 1) - 1) // (BRANCH - 1)  # 299593

# Device per-core x layout: [x4 (512) | x5 (4096, j-major) | x6 (32768,
# per-chunk block layout)]
X4_OFF = 0
X5_OFF = 512
X6_OFF = 512 + 4096
SUB_N = 512 + 4096 + 32768  # 37376
N_CHUNKS = 8  # leaf chunks: 512 L5-parents (4096 leaves) each

# tanh(x) ~= x*(PA + PB*x^2 + PD*x^4) on [-1, 1], max abs err 3.9e-4
PA = 0.99716306
PB = -0.30798629
PD = 0.07280493

LAST_RESULTS = None  # stash for test harness introspection


def _build_subtree_kernel():
    nc = bacc.Bacc("TRN2", target_bir_lowering=False, debug=False, num_devices=8)

    xs = nc.dram_tensor("xs", [128, SUB_N], F16, kind="ExternalInput").ap()
    wioux_d = nc.dram_tensor("wioux", [128, 384], F16, kind="ExternalInput").ap()
    wiouh_d = nc.dram_tensor("wiouh", [128, 384], F16, kind="ExternalInput").ap()
    wfx_d = nc.dram_tensor("wfx", [128, 128], F16, kind="ExternalInput").ap()
    wfh_d = nc.dram_tensor("wfh", [128, 128], F16, kind="ExternalInput").ap()
    ident_d = nc.dram_tensor("ident", [128, 128], F16, kind="ExternalInput").ap()
    biou_d = nc.dram_tensor("biou", [128, 3], F32, kind="ExternalInput").ap()
    biourow_d = nc.dram_tensor("biourow", [1, 384], F16, kind="ExternalInput").ap()
    bf_d = nc.dram_tensor("bf", [128, 1], F32, kind="ExternalInput").ap()
    out_d = nc.dram_tensor("out", [128, 1024], F16, kind="ExternalOutput").ap()

    with tile.TileContext(nc) as tc:
        with (
            tc.tile_pool(name="const", bufs=1) as cp,
            tc.tile_pool(name="xlo", bufs=1) as xlo_p,
            tc.tile_pool(name="x6", bufs=3) as x6_p,
            tc.tile_pool(name="leafst", bufs=2) as lf_p,
            tc.tile_pool(name="prodp", bufs=2) as prod_p,
            tc.tile_pool(name="c2p", bufs=2) as c2_p,
            tc.tile_pool(name="state", bufs=1) as st,
            tc.tile_pool(name="gates", bufs=2) as gp,
            tc.tile_pool(name="small", bufs=2) as sp,
            tc.tile_pool(name="tail", bufs=1) as tp,
            tc.tile_pool(name="psum", bufs=2, space="PSUM") as psum,
        ):
            # Dummy 1-col activation at t=0 so the ~1.3us ACT_TABLE_LOAD for
            # the sigmoid/tanh set overlaps the first DMAs.
            dum = cp.tile([128, 1], F16, tag="dum")
            nc.vector.memset(dum[:], 0.0)
            dumo = cp.tile([128, 1], F16, tag="dumo")
            nc.scalar.activation(dumo[:], dum[:], SIG)

            W = {}

            def load_const(name, dram, shape, dt, engine):
                t = cp.tile(shape, dt, tag=name)
                engine.dma_start(t[:], dram)
                W[name] = t

            # first leaf chunk half ASAP on the fast queue
            x6_tiles = {}
            x6_tiles[(0, 0)] = x6_p.tile([128, 2048], F16, tag="x6", name="x6_0_0")
            nc.sync.dma_start(x6_tiles[(0, 0)][:], xs[:, X6_OFF : X6_OFF + 2048])
            load_const("wioux", wioux_d, [128, 384], F16, nc.sync)
            load_const("biou", biou_d, [128, 3], F32, nc.sync)

            load_const("wiouh", wiouh_d, [128, 384], F16, nc.gpsimd)
            load_const("wfx", wfx_d, [128, 128], F16, nc.gpsimd)
            load_const("wfh", wfh_d, [128, 128], F16, nc.gpsimd)
            load_const("ident", ident_d, [128, 128], F16, nc.gpsimd)
            load_const("biourow", biourow_d, [1, 384], F16, nc.gpsimd)
            load_const("bf", bf_d, [128, 1], F32, nc.gpsimd)
            ones = cp.tile([1, 512], F16, tag="ones")
            nc.vector.memset(ones[:], 1.0)
            W["ones"] = ones

            x6_tiles[(0, 1)] = x6_p.tile([128, 2048], F16, tag="x6", name="x6_0_1")
            nc.sync.dma_start(x6_tiles[(0, 1)][:], xs[:, X6_OFF + 2048 : X6_OFF + 4096])

            # x for L4 + L5 (cols 0..4608), persistent
            x15 = xlo_p.tile([128, X6_OFF], F16)
            nc.sync.dma_start(x15[:], xs[:, 0:X6_OFF])

            def x5c(ch):
                return x15[:, X5_OFF + ch * 512 : X5_OFF + (ch + 1) * 512]

            x4 = x15[:, 0:512]

            # persistent state
            hs5 = st.tile([128, 4096], F16, tag="hs5")
            fc5 = st.tile([128, 4096], F16, tag="fc5")
            c5 = st.tile([128, 4096], F16, tag="c5")
            h5 = st.tile([128, 4096], F16, tag="h5")
            prod4 = st.tile([128, 4096], F16, tag="prod4")
            out_t = st.tile([128, 1024], F16, tag="out_t")

            leaf_states = {}
            sio5_tiles = {}

            def leaf_iou_half(ch, g):
                """iou gates for 2048 leaves (blocks 4g..4g+3 of chunk ch)."""
                if (ch, g) not in x6_tiles:
                    t = x6_p.tile([128, 2048], F16, tag="x6", name=f"x6_{ch}_{g}")
                    base = X6_OFF + ch * 4096 + g * 2048
                    nc.sync.dma_start(t[:], xs[:, base : base + 2048])
                    x6_tiles[(ch, g)] = t
                x6t = x6_tiles[(ch, g)]
                if ch not in leaf_states:
                    lc = lf_p.tile([128, 4096], F16, tag="lc", name=f"lc{ch}")
                    lh = lf_p.tile([128, 4096], F16, tag="lh", name=f"lh{ch}")
                    leaf_states[ch] = (lc, lh)
                lc, lh = leaf_states[ch]
                sl = slice(g * 2048, (g + 1) * 2048)

                def gate_psum(gate, name):
                    p = psum.tile([128, 2048], F32, tag="pg", name=name)
                    w = W["wioux"][:, gate * 128 : (gate + 1) * 128]
                    for s in range(0, 2048, 512):
                        nc.tensor.matmul(
                            p[:, s : s + 512], w, x6t[:, s : s + 512],
                            start=True, stop=True,
                        )
                    return p

                pi = gate_psum(0, f"pi6_{ch}_{g}")
                pu = gate_psum(2, f"pu6_{ch}_{g}")
                si = gp.tile([128, 2048], F16, tag="si", name=f"si6_{ch}_{g}")
                nc.scalar.activation(si[:], pi[:], SIG, bias=W["biou"][:, 0:1])
                tu = gp.tile([128, 2048], F16, tag="tu", name=f"tu6_{ch}_{g}")
                nc.scalar.activation(tu[:], pu[:], TANH, bias=W["biou"][:, 2:3])
                po = gate_psum(1, f"po6_{ch}_{g}")
                nc.vector.tensor_mul(lc[:, sl], si[:], tu[:])
                c2 = c2_p.tile([128, 2048], F16, tag="c2", name=f"c2_{ch}_{g}")
                nc.gpsimd.tensor_mul(c2[:], lc[:, sl], lc[:, sl])
                so = gp.tile([128, 2048], F16, tag="so", name=f"so6_{ch}_{g}")
                nc.scalar.activation(so[:], po[:], SIG, bias=W["biou"][:, 1:2])
                s = gp.tile([128, 2048], F16, tag="s", name=f"s6_{ch}_{g}")
                nc.vector.tensor_mul(s[:], so[:], lc[:, sl])
                return c2, s

            def leaf_poly_half(ch, g, c2, s):
                """h = s * (PA + c2*(PB + PD*c2)) for half g of chunk ch."""
                _, lh = leaf_states[ch]
                sl = slice(g * 2048, (g + 1) * 2048)
                q = gp.tile([128, 2048], F16, tag="q", name=f"q6_{ch}_{g}")
                nc.vector.tensor_scalar(q[:], c2[:], PD, PB, MULT, ADD)
                r = gp.tile([128, 2048], F16, tag="r", name=f"r6_{ch}_{g}")
                nc.vector.tensor_mul(r[:], q[:], c2[:])
                nc.vector.scalar_tensor_tensor(lh[:, sl], r[:], PA, s[:], ADD, MULT)

            def fgates(ch):
                """L5 forget gates + child-sums for chunk ch (4096 leaves)."""
                lc, lh = leaf_states.pop(ch)
                prods = []
                for g in range(2):
                    pf = psum.tile([128, 2048], F32, tag="pg", name=f"pf_{ch}_{g}")
                    for b in range(4):
                        jj = 4 * g + b
                        nc.tensor.matmul(
                            pf[:, b * 512 : (b + 1) * 512], W["wfh"][:],
                            lh[:, jj * 512 : (jj + 1) * 512],
                            start=True, stop=False,
                        )
                    for b in range(4):
                        nc.tensor.matmul(
                            pf[:, b * 512 : (b + 1) * 512], W["wfx"][:], x5c(ch),
                            start=False, stop=True,
                        )
                    f = gp.tile([128, 2048], F16, tag="f", name=f"f_{ch}_{g}")
                    nc.scalar.activation(f[:], pf[:], SIG, bias=W["bf"][:])
                    prod = prod_p.tile(
                        [128, 2048], F16, tag="prod", name=f"prod_{ch}_{g}"
                    )
                    nc.vector.tensor_mul(prod[:], f[:], lc[:, g * 2048 : (g + 1) * 2048])
                    prods.append(prod)
                # child-sums via identity-weight matmul accumulation
                pfchs = psum.tile([128, 1024], F32, tag="pg", name=f"pfchs_{ch}")
                for g in range(2):
                    for b in range(4):
                        jj = 4 * g + b
                        nc.tensor.matmul(
                            pfchs[:, 0:512], W["ident"][:],
                            prods[g][:, b * 512 : (b + 1) * 512],
                            start=(jj == 0), stop=(jj == 7),
                        )
                for jj in range(8):
                    nc.tensor.matmul(
                        pfchs[:, 512:1024], W["ident"][:],
                        lh[:, jj * 512 : (jj + 1) * 512],
                        start=(jj == 0), stop=(jj == 7),
                    )
                cols = slice(ch * 512, (ch + 1) * 512)
                nc.vector.tensor_copy(fc5[:, cols], pfchs[:, 0:512])
                nc.vector.tensor_copy(hs5[:, cols], pfchs[:, 512:1024])

            def l5iou(ch):
                """iou + cell update for the 512 L5 nodes of chunk ch."""
                cols = slice(ch * 512, (ch + 1) * 512)
                p = psum.tile([128, 1536], F32, tag="pg", name=f"pio5_{ch}")
                for gate in range(3):
                    sl = slice(gate * 512, (gate + 1) * 512)
                    w = W["wioux"][:, gate * 128 : (gate + 1) * 128]
                    nc.tensor.matmul(p[:, sl], w, x5c(ch), start=True, stop=False)
                for gate in range(3):
                    sl = slice(gate * 512, (gate + 1) * 512)
                    wh = W["wiouh"][:, gate * 128 : (gate + 1) * 128]
                    nc.tensor.matmul(p[:, sl], wh, hs5[:, cols], start=False, stop=False)
                for gate in range(3):
                    sl = slice(gate * 512, (gate + 1) * 512)
                    br = W["biourow"][:, gate * 128 : (gate + 1) * 128]
                    nc.tensor.matmul(p[:, sl], br, W["ones"][:, 0:512], start=False, stop=True)
                sio = sp.tile([128, 1024], F16, tag="sio", name=f"sio5_{ch}")
                nc.scalar.activation(sio[:], p[:, 0:1024], SIG)
                tu = sp.tile([128, 512], F16, tag="tu5", name=f"tu5_{ch}")
                nc.scalar.activation(tu[:], p[:, 1024:1536], TANH)
                ct = sp.tile([128, 512], F16, tag="ct5", name=f"ct5_{ch}")
                nc.vector.tensor_mul(ct[:], sio[:, 0:512], tu[:])
                nc.vector.tensor_add(c5[:, cols], ct[:], fc5[:, cols])
                tct = sp.tile([128, 512], F16, tag="tct5", name=f"tct5_{ch}")
                nc.scalar.activation(tct[:], c5[:, cols], TANH)
                nc.vector.tensor_mul(h5[:, cols], sio[:, 512:1024], tct[:])

            def l4f(b):
                """L4 forget gates for child-block b (= L5 storage block b)."""
                bcols = slice(b * 512, (b + 1) * 512)
                pf = psum.tile([128, 512], F32, tag="pg", name=f"pf4_{b}")
                nc.tensor.matmul(pf[:], W["wfh"][:], h5[:, bcols], start=True, stop=False)
                nc.tensor.matmul(pf[:], W["wfx"][:], x4, start=False, stop=True)
                f = sp.tile([128, 512], F16, tag="f4", name=f"f4_{b}")
                nc.scalar.activation(f[:], pf[:], SIG, bias=W["bf"][:])
                nc.vector.tensor_mul(prod4[:, bcols], f[:], c5[:, bcols])

            # ---- main pipeline ----
            for ch in range(N_CHUNKS):
                cs0 = leaf_iou_half(ch, 0)
                cs1 = leaf_iou_half(ch, 1)
                leaf_poly_half(ch, 0, *cs0)
                leaf_poly_half(ch, 1, *cs1)
                if ch >= 1:
                    fgates(ch - 1)
                if ch >= 2:
                    l5iou(ch - 2)
                if ch >= 3:
                    l4f(ch - 3)

            fgates(7)
            l5iou(6)
            l4f(5)
            l5iou(7)
            l4f(6)
            l4f(7)

            # ---- L4 child-sum trees (contiguous fp16 adds) + iou/top ----
            t1f = tp.tile([128, 2048], F16, tag="t1", name="t1f")
            nc.vector.tensor_add(t1f[:], prod4[:, 0:2048], prod4[:, 2048:4096])
            t2f = tp.tile([128, 1024], F16, tag="t2", name="t2f")
            nc.vector.tensor_add(t2f[:], t1f[:, 0:1024], t1f[:, 1024:2048])
            fc4 = st.tile([128, 512], F16, tag="fc4")
            nc.vector.tensor_add(fc4[:], t2f[:, 0:512], t2f[:, 512:1024])
            t1h = tp.tile([128, 2048], F16, tag="t1", name="t1h")
            nc.vector.tensor_add(t1h[:], h5[:, 0:2048], h5[:, 2048:4096])
            t2h = tp.tile([128, 1024], F16, tag="t2", name="t2h")
            nc.vector.tensor_add(t2h[:], t1h[:, 0:1024], t1h[:, 1024:2048])
            hs4 = st.tile([128, 512], F16, tag="hs4")
            nc.vector.tensor_add(hs4[:], t2h[:, 0:512], t2h[:, 512:1024])

            p4 = psum.tile([128, 1536], F32, tag="pg", name="pio4")
            for gate in range(3):
                sl = slice(gate * 512, (gate + 1) * 512)
                w = W["wioux"][:, gate * 128 : (gate + 1) * 128]
                nc.tensor.matmul(p4[:, sl], w, x4, start=True, stop=False)
            for gate in range(3):
                sl = slice(gate * 512, (gate + 1) * 512)
                wh = W["wiouh"][:, gate * 128 : (gate + 1) * 128]
                nc.tensor.matmul(p4[:, sl], wh, hs4[:], start=False, stop=False)
            for gate in range(3):
                sl = slice(gate * 512, (gate + 1) * 512)
                br = W["biourow"][:, gate * 128 : (gate + 1) * 128]
                nc.tensor.matmul(p4[:, sl], br, W["ones"][:, 0:512], start=False, stop=True)
            sio4 = tp.tile([128, 1024], F16, tag="sio4")
            nc.scalar.activation(sio4[:], p4[:, 0:1024], SIG)
            tu4 = tp.tile([128, 512], F16, tag="tu4")
            nc.scalar.activation(tu4[:], p4[:, 1024:1536], TANH)
            ct4 = tp.tile([128, 512], F16, tag="ct4")
            nc.vector.tensor_mul(ct4[:], sio4[:, 0:512], tu4[:])
            nc.vector.tensor_add(out_t[:, 0:512], ct4[:], fc4[:])
            tct4 = tp.tile([128, 512], F16, tag="tct4")
            nc.scalar.activation(tct4[:], out_t[:, 0:512], TANH)
            nc.vector.tensor_mul(out_t[:, 512:1024], sio4[:, 512:1024], tct4[:])
            nc.sync.dma_start(out_d, out_t[:])

    nc.compile()
    return nc


_NC_CACHE = None


def _get_nc():
    global _NC_CACHE
    if _NC_CACHE is None:
        _NC_CACHE = _build_subtree_kernel()
    return _NC_CACHE


def _sigmoid(x):
    return 1.0 / (1.0 + np.exp(-x))


def _perm_x5():
    # device x5 col j*512 + q  <-  local L5 node 8q + j
    j, q = np.meshgrid(np.arange(8), np.arange(512), indexing="ij")
    return (8 * q + j).reshape(-1)


def _perm_x6():
    # device x6 col ch*4096 + jj*512 + m  <-  local leaf 64m + 8ch + jj
    ch, jj, m = np.meshgrid(
        np.arange(8), np.arange(8), np.arange(512), indexing="ij"
    )
    return (64 * m + 8 * ch + jj).reshape(-1)


def kernel(
    x, W_ioux, b_ioux, W_iouh, b_iouh, W_fx, b_fx, W_fh, b_fh, branch, depth
):
    global LAST_RESULTS
    assert int(branch) == BRANCH and int(depth) == DEPTH

    x = np.asarray(x, np.float32)
    W_ioux = np.asarray(W_ioux, np.float32)
    b_ioux = np.asarray(b_ioux, np.float32)
    W_iouh = np.asarray(W_iouh, np.float32)
    b_iouh = np.asarray(b_iouh, np.float32)
    W_fx = np.asarray(W_fx, np.float32)
    b_fx = np.asarray(b_fx, np.float32)
    W_fh = np.asarray(W_fh, np.float32)
    b_fh = np.asarray(b_fh, np.float32)

    wioux = np.ascontiguousarray(W_ioux.T.astype(np.float16))
    wiouh = np.ascontiguousarray(W_iouh.T.astype(np.float16))
    wfx = np.ascontiguousarray(W_fx.T.astype(np.float16))
    wfh = np.ascontiguousarray(W_fh.T.astype(np.float16))
    ident = np.eye(128, dtype=np.float16)
    biou_full = b_ioux + b_iouh
    biou = np.ascontiguousarray(biou_full.reshape(3, 128).T.astype(np.float32))
    biourow = np.ascontiguousarray(biou_full.reshape(1, 384).astype(np.float16))
    bf = np.ascontiguousarray((b_fx + b_fh).reshape(128, 1).astype(np.float32))

    off = lambda l: (BRANCH**l - 1) // (BRANCH - 1)
    p5, p6 = _perm_x5(), _perm_x6()
    in_maps = []
    for c in range(BRANCH):
        x4_c = x[off(4) + c * 512 : off(4) + (c + 1) * 512]
        x5_c = x[off(5) + c * 4096 : off(5) + (c + 1) * 4096][p5]
        x6_c = x[off(6) + c * 32768 : off(6) + (c + 1) * 32768][p6]
        xs_c = np.ascontiguousarray(
            np.concatenate([x4_c, x5_c, x6_c], axis=0).T.astype(np.float16)
        )
        in_maps.append(
            {
                "xs": xs_c,
                "wioux": wioux,
                "wiouh": wiouh,
                "wfx": wfx,
                "wfh": wfh,
                "ident": ident,
                "biou": biou,
                "biourow": biourow,
                "bf": bf,
            }
        )

    nc = _get_nc()
    trace = os.environ.get("TREELSTM_TRACE") == "1"
    res = run_bass_kernel_spmd(nc, in_maps, core_ids=list(range(8)), trace=trace)
    LAST_RESULTS = res

    # device out: [128, 1024] fp16 = [c4 (512) | h4 (512)] per core
    c_prev = np.concatenate(
        [res.results[c]["out"][:, 0:512].T.astype(np.float32) for c in range(8)]
    )  # [4096, 128] = global level 4, BFS order
    h_prev = np.concatenate(
        [res.results[c]["out"][:, 512:1024].T.astype(np.float32) for c in range(8)]
    )

    # Host: levels 3..0 exactly as the reference (fp32).
    for l in range(3, -1, -1):
        n_l = BRANCH**l
        x_l = x[off(l) : off(l) + n_l]
        c_ch = c_prev.reshape(n_l, BRANCH, MEM)
        h_ch = h_prev.reshape(n_l, BRANCH, MEM)
        h_sum = h_ch.sum(axis=1)
        f = _sigmoid(
            np.einsum("nbm,km->nbk", h_ch, W_fh) + b_fh
            + (x_l @ W_fx.T + b_fx)[:, None, :]
        )
        fc_sum = (f * c_ch).sum(axis=1)
        iou = x_l @ W_ioux.T + b_ioux + h_sum @ W_iouh.T + b_iouh
        i, o, u = iou[:, :MEM], iou[:, MEM : 2 * MEM], iou[:, 2 * MEM :]
        c_prev = _sigmoid(i) * np.tanh(u) + fc_sum
        h_prev = _sigmoid(o) * np.tanh(c_prev)
    return (c_prev.astype(np.float32), h_prev.astype(np.float32))
